# revision 9
# baseline (speedup 1.0000x reference)
"""Trainium2 Bass kernel for the Consis_Reg MSE loss.

Reference semantics (N=8192, D=512, C=64 classes):
    S[i,j]    = ||a_i - a_j||^2
    per_row_i = sum_{j: t_j == t_i} S[i,j] / cnt_{t_i}
    loss      = sum_i per_row_i

Class-aggregation identity (exact in real arithmetic):
    loss = 2 * ( total_sumsq - sum_c ||sumA_c||^2 / cnt_c )
where sumA_c = sum_{i in c} a_i, cnt_c = |{i: t_i == c}|,
total_sumsq = sum_i ||a_i||^2.

Device work per core (1024-row shard), A staged as fp8 e4m3
(quantization shifts the loss by ~7e-4 relative — far inside the 2e-2
gate — and quarters the HBM traffic):
    osum [64, 512] bf16 = sum_r M_r^T @ A_r   (4 DoubleRow fp8 matmuls,
                                               PSUM f32 accumulation)
    osq  [1, 3] f32     = sum of squares, split across DVE / Scalar /
                          GpSimd into per-partition f32 accumulators,
                          then a ones-vector matmul folds the partition
                          dim so the output DMA is a single descriptor
The one-hot M is built on-device (iota + is_equal) from the tiny
targets tensor, which is dispatched before A and lands while A is
still on the wire — M never sits on the critical path. Class counts
are a host-side bincount of targets (part of the partial combine,
like the cross-core sum itself).

DMA notes (measured): descriptor generation is the stream bottleneck
(~4ns + ~3.5ns/KB per partition-row descriptor, ~2x faster when
several dispatches are queued on the ring), so A is split into two
pipelined chunks on the SP ring and the matmul/sumsq consumers chase
the chunk semaphores; outputs split across the SP and Activation
rings, whose descriptor generators run concurrently.
"""

import numpy as np
import ml_dtypes

N, D, C = 8192, 512, 64
NCORES = 8
ROWS = N // NCORES  # rows per core
P = 128             # SBUF partitions
NT = ROWS // P      # row-tiles per core (rows per partition)

F8 = ml_dtypes.float8_e4m3  # matches TRN FP8_EXP4 encoding for |x| <= 240

_PROGRAM_CACHE = {}


def _build_program():
    import concourse.bass as bass
    import concourse.bacc as bacc
    import concourse.tile as tile
    from concourse import mybir

    f32 = mybir.dt.float32
    bf16 = mybir.dt.bfloat16
    f8 = mybir.dt.float8e4
    u8 = mybir.dt.uint8
    i32 = mybir.dt.int32
    HB = NT * D // 2  # bytes per partition per A chunk (2048)

    nc = bacc.Bacc(
        "TRN2", target_bir_lowering=False, debug=False, num_devices=NCORES
    )
    t_dram = nc.dram_tensor("t", [P, NT], i32, kind="ExternalInput").ap()
    alo_dram = nc.dram_tensor("alo", [P, HB], u8, kind="ExternalInput").ap()
    ahi_dram = nc.dram_tensor("ahi", [P, HB], u8, kind="ExternalInput").ap()
    osum_lo = nc.dram_tensor("osum_lo", [C // 2, D], bf16, kind="ExternalOutput").ap()
    osum_hi = nc.dram_tensor("osum_hi", [C // 2, D], bf16, kind="ExternalOutput").ap()
    osq = nc.dram_tensor("osq", [1, 2], f32, kind="ExternalOutput").ap()

    with tile.TileContext(nc) as tc:
        with (
            tc.tile_pool(name="big", bufs=1) as big,
            tc.tile_pool(name="small", bufs=1) as small,
            tc.tile_pool(name="psum", bufs=1, space="PSUM") as pspool,
        ):
            # SP ring FIFO: tiny t first (cheap descriptors, early land),
            # then the two pipelined A chunks
            t_sb = small.tile([P, NT], i32)
            nc.sync.dma_start(out=t_sb, in_=t_dram)
            alo_sb = big.tile([P, HB], u8, tag="alo")
            nc.sync.dma_start(out=alo_sb, in_=alo_dram)
            ahi_sb = big.tile([P, HB], u8, tag="ahi")
            nc.sync.dma_start(out=ahi_sb, in_=ahi_dram)

            alo8 = alo_sb.bitcast(f8)
            ahi8 = ahi_sb.bitcast(f8)
            alo_r = alo8.rearrange("p (a d) -> p a d", a=NT // 2)
            ahi_r = ahi8.rearrange("p (a d) -> p a d", a=NT // 2)

            # one-hot M in fp8 (0/1 exact): iota has no deps, the compare
            # only needs the tiny t tensor
            iota_f = small.tile([P, NT, C], f32)
            nc.gpsimd.iota(
                iota_f,
                pattern=[[0, NT], [1, C]],
                base=0,
                channel_multiplier=0,
                allow_small_or_imprecise_dtypes=True,
            )
            t_f = small.tile([P, NT], f32)
            nc.vector.tensor_copy(t_f, t_sb)
            t_b = bass.AP(
                tensor=t_f.tensor,
                offset=t_f.offset,
                ap=[t_f.ap[0], t_f.ap[1], [0, C]],
            )
            m_sb = small.tile([P, NT, C], f8)
            nc.vector.tensor_tensor(
                m_sb, iota_f, t_b, mybir.AluOpType.is_equal
            )

            # 4 DoubleRow matmuls: pair k contracts row-tiles 2k, 2k+1;
            # pairs 0,1 chase the alo chunk, pairs 2,3 the ahi chunk
            psum_s = pspool.tile([C, D], f32)
            for k in range(4):
                src = alo_r if k < 2 else ahi_r
                r = (2 * k) % 4
                nc.tensor.matmul(
                    psum_s,
                    lhsT=m_sb[:, 2 * k : 2 * k + 2, :],
                    rhs=src[:, r : r + 2, :],
                    start=(k == 0),
                    stop=(k == 3),
                    perf_mode=mybir.MatmulPerfMode.DoubleRow,
                )

            # sum of squares: DVE takes all of alo (lands first), Scalar
            # takes all of ahi
            sqp = small.tile([P, 2], f32)
            scr0 = big.tile([P, HB], bf16, tag="scr0")
            nc.vector.scalar_tensor_tensor(
                out=scr0,
                in0=alo8,
                scalar=1.0,
                in1=alo8,
                op0=mybir.AluOpType.mult,
                op1=mybir.AluOpType.mult,
                accum_out=sqp[:, 0:1],
            )
            scr1 = big.tile([P, HB], bf16, tag="scr1")
            nc.scalar.activation(
                scr1,
                ahi8,
                mybir.ActivationFunctionType.Square,
                accum_out=sqp[:, 1:2],
            )

            # class sums: PSUM -> SBUF (bf16), then halves out on the two
            # rings (descriptor generators run concurrently)
            osum_sb = small.tile([C, D], bf16)
            nc.vector.tensor_copy(osum_sb, psum_s)
            nc.sync.dma_start(out=osum_lo, in_=osum_sb[0:32, :])
            nc.scalar.dma_start(out=osum_hi, in_=osum_sb[32:64, :])

            # fold sumsq partials across partitions: ones^T @ sqp -> [1, 3]
            ones = nc.const_aps.aps[(f32, 1.0)]
            psum_q = pspool.tile([1, 2], f32)
            nc.tensor.matmul(psum_q, lhsT=ones, rhs=sqp[:], start=True, stop=True)
            osq_sb = small.tile([1, 2], f32)
            nc.vector.tensor_copy(osq_sb, psum_q)
            nc.sync.dma_start(out=osq, in_=osq_sb)

    nc.compile()
    return nc


def get_program():
    if "nc" not in _PROGRAM_CACHE:
        _PROGRAM_CACHE["nc"] = _build_program()
    return _PROGRAM_CACHE["nc"]


def make_in_maps(representations, targets):
    A = np.asarray(representations, dtype=np.float32)
    t = np.asarray(targets).astype(np.int32)
    A8 = A.astype(F8)  # [N, D] fp8
    HB = NT * D // 2
    in_maps = []
    for core in range(NCORES):
        sl = slice(core * ROWS, (core + 1) * ROWS)
        a_u8 = A8[sl].view(np.uint8).reshape(P, NT * D)
        in_maps.append({
            "t": np.ascontiguousarray(t[sl].reshape(P, NT)),
            "alo": np.ascontiguousarray(a_u8[:, :HB]),
            "ahi": np.ascontiguousarray(a_u8[:, HB:]),
        })
    return in_maps


def combine_partials(results, targets):
    cnt = np.bincount(np.asarray(targets).astype(np.int64), minlength=C)
    sums = np.zeros((C, D), np.float64)
    total_sumsq = 0.0
    for r in results:
        sums[: C // 2] += np.asarray(r["osum_lo"]).astype(np.float64)
        sums[C // 2 :] += np.asarray(r["osum_hi"]).astype(np.float64)
        total_sumsq += float(np.asarray(r["osq"]).astype(np.float64).sum())
    loss = 2.0 * (
        total_sumsq - ((sums * sums).sum(axis=1) / cnt).sum()
    )
    return np.float32(loss)


def kernel(representations, targets):
    from concourse.bass_utils import run_bass_kernel_spmd

    nc = get_program()
    in_maps = make_in_maps(representations, targets)
    res = run_bass_kernel_spmd(nc, in_maps, list(range(NCORES)))
    return combine_partials(res.results, targets)


# revision 10
# speedup vs baseline: 1.0808x; 1.0808x over previous
"""Trainium2 Bass kernel for the Consis_Reg MSE loss.

Reference semantics (N=8192, D=512, C=64 classes):
    S[i,j]    = ||a_i - a_j||^2
    per_row_i = sum_{j: t_j == t_i} S[i,j] / cnt_{t_i}
    loss      = sum_i per_row_i

Class-aggregation identity (exact in real arithmetic):
    loss = 2 * ( total_sumsq - sum_c ||sumA_c||^2 / cnt_c )
where sumA_c = sum_{i in c} a_i, cnt_c = |{i: t_i == c}|,
total_sumsq = sum_i ||a_i||^2.

Device work per core (1024-row shard), inputs staged as fp8 e4m3
(quantization shifts the loss by ~7e-4 relative — far inside the 2e-2
gate — and quarters the HBM traffic):
    osum [64, 512] bf16 = sum_r M_r^T @ A_r   (4 DoubleRow fp8 matmuls,
                                               PSUM f32 accumulation)
    osq  [1, 2] f32     = sum of squares, DVE half + Scalar half into
                          per-partition f32 accumulators, then a
                          ones-vector matmul folds the partition dim so
                          the output DMA is a single descriptor
The one-hot M is built on the host (fp8 0/1 is exact) and packed at
the head of each partition's input row — no iota/compare on device.
Class counts are a host-side bincount of targets (part of the partial
combine, like the cross-core sum itself).

DMA notes (measured): the runtime descriptor generator is the stream
bottleneck (~15-19ns per partition-row descriptor; queueing more
dispatches on a ring interleaves their descriptors and delays every
completion), so the input is ONE dispatch of 128 descriptors, and the
osum output is split in half across the SP and Activation rings,
whose generators run concurrently.
"""

import numpy as np
import ml_dtypes

N, D, C = 8192, 512, 64
NCORES = 8
ROWS = N // NCORES  # rows per core
P = 128             # SBUF partitions
NT = ROWS // P      # row-tiles per core (rows per partition)

F8 = ml_dtypes.float8_e4m3  # matches TRN FP8_EXP4 encoding for |x| <= 240

_PROGRAM_CACHE = {}


def _build_program():
    import concourse.bass as bass
    import concourse.bacc as bacc
    import concourse.tile as tile
    from concourse import mybir

    f32 = mybir.dt.float32
    bf16 = mybir.dt.bfloat16
    f8 = mybir.dt.float8e4
    u8 = mybir.dt.uint8
    ROW = 512 + NT * D  # 4608 bytes per partition: M row block + A row block

    nc = bacc.Bacc(
        "TRN2", target_bir_lowering=False, debug=False, num_devices=NCORES
    )
    ind = nc.dram_tensor("ind", [P, ROW], u8, kind="ExternalInput").ap()
    osum_lo = nc.dram_tensor("osum_lo", [C // 2, D], bf16, kind="ExternalOutput").ap()
    osum_hi = nc.dram_tensor("osum_hi", [C // 2, D], bf16, kind="ExternalOutput").ap()
    osq = nc.dram_tensor("osq", [1, 2], f32, kind="ExternalOutput").ap()

    with tile.TileContext(nc) as tc:
        with (
            tc.tile_pool(name="big", bufs=1) as big,
            tc.tile_pool(name="small", bufs=1) as small,
            tc.tile_pool(name="psum", bufs=1, space="PSUM") as pspool,
        ):
            in_sb = big.tile([P, ROW], u8, tag="in")
            nc.sync.dma_start(out=in_sb, in_=ind)

            m_ap = in_sb[:, 0:512].bitcast(f8).rearrange(
                "p (a c) -> p a c", a=NT
            )
            a_ap = in_sb[:, 512:ROW].bitcast(f8).rearrange(
                "p (a d) -> p a d", a=NT
            )
            av = in_sb[:, 512:ROW].bitcast(f8)

            # 4 DoubleRow matmuls: pair k contracts row-tiles 2k, 2k+1
            psum_s = pspool.tile([C, D], f32)
            for k in range(4):
                nc.tensor.matmul(
                    psum_s,
                    lhsT=m_ap[:, 2 * k : 2 * k + 2, :],
                    rhs=a_ap[:, 2 * k : 2 * k + 2, :],
                    start=(k == 0),
                    stop=(k == 3),
                    perf_mode=mybir.MatmulPerfMode.DoubleRow,
                )

            # sum of squares: DVE and Scalar split the elements; the DVE
            # gets the smaller share so it frees up for the PSUM copy
            SPLIT = 1664
            sqp = small.tile([P, 2], f32)
            scr0 = big.tile([P, SPLIT], bf16, tag="scr0")
            nc.vector.scalar_tensor_tensor(
                out=scr0,
                in0=av[:, 0:SPLIT],
                scalar=1.0,
                in1=av[:, 0:SPLIT],
                op0=mybir.AluOpType.mult,
                op1=mybir.AluOpType.mult,
                accum_out=sqp[:, 0:1],
            )
            scr1 = big.tile([P, 4096 - SPLIT], bf16, tag="scr1")
            nc.scalar.activation(
                scr1,
                av[:, SPLIT:4096],
                mybir.ActivationFunctionType.Square,
                accum_out=sqp[:, 1:2],
            )

            # class sums: PSUM -> SBUF (bf16), halves out on the two rings
            # (their descriptor generators run concurrently)
            osum_sb = small.tile([C, D], bf16)
            nc.vector.tensor_copy(osum_sb, psum_s)
            nc.sync.dma_start(out=osum_lo, in_=osum_sb[0:32, :])
            nc.scalar.dma_start(out=osum_hi, in_=osum_sb[32:64, :])

            # fold sumsq partials across partitions: ones^T @ sqp -> [1, 2]
            ones = nc.const_aps.aps[(f32, 1.0)]
            psum_q = pspool.tile([1, 2], f32)
            nc.tensor.matmul(psum_q, lhsT=ones, rhs=sqp[:], start=True, stop=True)
            osq_sb = small.tile([1, 2], f32)
            nc.vector.tensor_copy(osq_sb, psum_q)
            nc.scalar.dma_start(out=osq, in_=osq_sb)

    nc.compile()
    return nc


def get_program():
    if "nc" not in _PROGRAM_CACHE:
        _PROGRAM_CACHE["nc"] = _build_program()
    return _PROGRAM_CACHE["nc"]


def make_in_maps(representations, targets):
    A = np.asarray(representations, dtype=np.float32)
    t = np.asarray(targets).astype(np.int64)
    A8 = A.astype(F8)                                      # [N, D] fp8
    M8 = (t[:, None] == np.arange(C)[None, :]).astype(F8)  # [N, C] fp8
    in_maps = []
    for core in range(NCORES):
        sl = slice(core * ROWS, (core + 1) * ROWS)
        a_u8 = A8[sl].view(np.uint8).reshape(P, NT * D)    # [128, 4096]
        m_u8 = M8[sl].view(np.uint8).reshape(P, NT * C)    # [128, 512]
        in_maps.append({"ind": np.concatenate([m_u8, a_u8], axis=1)})
    return in_maps


def combine_partials(results, targets):
    cnt = np.bincount(np.asarray(targets).astype(np.int64), minlength=C)
    sums = np.zeros((C, D), np.float64)
    total_sumsq = 0.0
    for r in results:
        sums[: C // 2] += np.asarray(r["osum_lo"]).astype(np.float64)
        sums[C // 2 :] += np.asarray(r["osum_hi"]).astype(np.float64)
        total_sumsq += float(np.asarray(r["osq"]).astype(np.float64).sum())
    loss = 2.0 * (
        total_sumsq - ((sums * sums).sum(axis=1) / cnt).sum()
    )
    return np.float32(loss)


def kernel(representations, targets):
    from concourse.bass_utils import run_bass_kernel_spmd

    nc = get_program()
    in_maps = make_in_maps(representations, targets)
    res = run_bass_kernel_spmd(nc, in_maps, list(range(NCORES)))
    return combine_partials(res.results, targets)
